# revision 28
# baseline (speedup 1.0000x reference)
"""DeepseekV2 MoE layer (T=256, H=2048, E=64, I=1408, top-6) on 8 TRN2 NeuronCores.

Strategy: expert-parallel. Each core owns 8 experts (w1/w2 shards), computes the
fp32 router for all 256 tokens (gate replicated; gate columns permuted per core
so the core's own experts land in columns 0..7 -> the SPMD program needs no
core id), runs the dense expert MLP for its 8 experts weighted by the routing
weights, and returns a partial [256, 2048] output. Host sums the 8 partials.

Expert MLP matmuls run in bf16 (PE ~315us ~= weight-DMA ~266us: the ridge;
~3.4e-3 rel err); the router runs in true float32 so top-k selection matches
the reference exactly. Measured ~359us NEFF exec per core.
"""
import os
import sys

sys.path.insert(0, "/opt/trn_rl_repo")

import numpy as np

import concourse.bass as bass
import concourse.mybir as mybir
import concourse.tile as tile
from concourse import bacc
from concourse.bass_utils import run_bass_kernel_spmd

# Content-hash NEFF cache: walrus takes minutes on this graph; identical BIR
# always yields an identical NEFF, so cache it across processes.
import hashlib
import shutil

import concourse.bass_utils as _bu
import concourse.bass2jax as _b2j

_orig_compile_bir = _bu.compile_bir_kernel


def _cached_compile_bir(bir_json, tmpdir, neff_name="file.neff"):
    cdir = "/root/.bass_neff_cache"
    os.makedirs(cdir, exist_ok=True)
    cpath = os.path.join(cdir, hashlib.sha256(bir_json).hexdigest()[:24] + ".neff")
    if os.path.exists(cpath):
        dst = os.path.join(tmpdir, neff_name)
        shutil.copyfile(cpath, dst)
        return dst
    p = _orig_compile_bir(bir_json, tmpdir, neff_name)
    shutil.copyfile(p, cpath + ".tmp")
    os.replace(cpath + ".tmp", cpath)
    return p


_bu.compile_bir_kernel = _cached_compile_bir
_b2j.compile_bir_kernel = _cached_compile_bir

T, H, E, I, TOPK = 256, 2048, 64, 1408, 6
NCORES = 8
EL = E // NCORES          # experts per core
HK = H // 128             # 16 k-tiles over hidden dim
IT = I // 128             # 11 i-tiles
NO = H // 512             # 4 output column tiles
IGROUPS = [(0, 4), (4, 4), (8, 3)]   # i-tile groups (PSUM-bank limited)
F32 = mybir.dt.float32

# bf16 expert MLP hits the compute/memory ridge (~365us vs ~630us for f32r)
# at rel err ~3.4e-3; set BASS_MOE_DTYPE=float32r for ~2.1e-4 if needed.
MDT = {
    "float32r": mybir.dt.float32r,
    "float32": mybir.dt.float32,
    "bfloat16": mybir.dt.bfloat16,
}[os.environ.get("BASS_MOE_DTYPE", "bfloat16")]


def _np_of(dt):
    if dt == mybir.dt.bfloat16:
        import ml_dtypes
        return ml_dtypes.bfloat16
    return np.float32


def build(mdt=MDT):
    nc = bacc.Bacc(None, target_bir_lowering=False)
    xt_d = nc.declare_dram_parameter("xt", [128, HK * T], mdt, isOutput=False)
    xt32_d = nc.declare_dram_parameter("xt32", [128, HK * T], F32, isOutput=False)
    gate_d = nc.declare_dram_parameter("gate", [128, HK * E], F32, isOutput=False)
    w1_d = nc.declare_dram_parameter("w1", [EL, H, I], mdt, isOutput=False)
    w2_d = nc.declare_dram_parameter("w2", [EL, I, H], mdt, isOutput=False)
    out_d = nc.declare_dram_parameter("out", [T, H], F32, isOutput=True)

    with tile.TileContext(nc) as tc:
        with (
            tc.tile_pool(name="const", bufs=1) as const,
            tc.tile_pool(name="rpool", bufs=2) as rpool,
            tc.tile_pool(name="w1pool", bufs=8) as w1pool,
            tc.tile_pool(name="w2pool", bufs=5) as w2pool,
            tc.tile_pool(name="hpool", bufs=2) as hpool,
            tc.tile_pool(name="psa", bufs=5, space="PSUM") as psa,
            tc.tile_pool(name="psb", bufs=2, space="PSUM") as psb,
            tc.tile_pool(name="psr", bufs=1, space="PSUM") as psr,
        ):
            # Warm both HWDGE rings + the DMA path with tiny transfers first.
            warm = const.tile([128, 8], F32, tag="warm")
            nc.sync.dma_start(out=warm[:, 0:1], in_=gate_d[:, 0:1])
            nc.scalar.dma_start(out=warm[:, 1:2], in_=gate_d[:, 1:2])

            # Warm the PE HAM clock gate during the DMA-bound head: ~4.5us of
            # junk matmuls so the real stream starts at 2.4GHz, not 1.2.
            warm_mm = const.tile([128, 8], F32, tag="warm_mm")
            nc.vector.memset(warm_mm, 0.0)
            ps_w = psr.tile([128, E], F32, tag="ps_r", name="ps_w")
            for _ in range(56):
                nc.tensor.matmul(ps_w[0:8, 0:8], lhsT=warm_mm, rhs=warm_mm,
                                 start=True, stop=True)

            # xt on the scalar ring so sync starts streaming w1 immediately
            xt_sb = const.tile([128, HK * T], mdt, tag="xt_sb")
            for hh in range(4):
                c0 = hh * 4 * T
                nc.scalar.dma_start(out=xt_sb[:, c0:c0 + 4 * T],
                                    in_=xt_d[:, c0:c0 + 4 * T])

            def emit_router_inputs():
                # scalar (ACT) HWDGE ring: off the critical w1 stream
                nc.scalar.dma_start(out=xt32_sb, in_=xt32_d[:, :])
                nc.scalar.dma_start(out=gate_sb, in_=gate_d[:, :])

            xt32_sb = const.tile([128, HK * T], F32, tag="xt32_sb")
            gate_sb = const.tile([128, HK * E], F32, tag="gate_sb")

            acc = []
            for tt in range(2):
                a = const.tile([128, H], F32, tag=f"acc{tt}")
                nc.vector.memset(a, 0.0)
                acc.append(a)

            # Anchor the warm-up matmuls against DCE: acc += 0 * ps_w (exact
            # no-op: ps_w is zeros and the scalar is 0.0).
            nc.vector.scalar_tensor_tensor(
                out=acc[0][:, 0:1], in0=ps_w[:, 0:1], scalar=0.0,
                in1=acc[0][:, 0:1], op0=mybir.AluOpType.mult,
                op1=mybir.AluOpType.add)

            # ---- router (true fp32) ----
            wf = []

            def emit_router(tt):
                ps_r = psr.tile([128, E], F32, tag="ps_r")
                for hk in range(HK):
                    c0 = hk * T + tt * 128
                    nc.tensor.matmul(
                        ps_r,
                        lhsT=xt32_sb[:, c0:c0 + 128],
                        rhs=gate_sb[:, hk * E:(hk + 1) * E],
                        start=hk == 0,
                        stop=hk == HK - 1,
                    )
                mx = rpool.tile([128, 1], F32, tag="mx")
                nc.vector.tensor_reduce(mx, ps_r, axis=mybir.AxisListType.X,
                                        op=mybir.AluOpType.max)
                negmax = rpool.tile([128, 1], F32, tag="negmax")
                nc.vector.tensor_scalar(negmax, mx, -1.0, None,
                                        op0=mybir.AluOpType.mult)
                exp_sb = rpool.tile([128, E], F32, tag="exp_sb")
                nc.scalar.activation(exp_sb, ps_r,
                                     mybir.ActivationFunctionType.Exp,
                                     bias=negmax)
                max8 = rpool.tile([128, 8], F32, tag="max8")
                nc.vector.max(max8, exp_sb)
                masked = rpool.tile([128, E], F32, tag="masked")
                nc.vector.scalar_tensor_tensor(
                    out=masked, in0=exp_sb, scalar=max8[:, TOPK - 1:TOPK],
                    in1=exp_sb, op0=mybir.AluOpType.is_ge,
                    op1=mybir.AluOpType.mult)
                ssum = rpool.tile([128, 1], F32, tag="ssum")
                nc.vector.reduce_sum(ssum, masked, axis=mybir.AxisListType.X)
                inv = rpool.tile([128, 1], F32, tag="inv")
                nc.vector.reciprocal(inv, ssum)
                w = rpool.tile([128, E], F32, tag=f"wf{tt}", name=f"wf{tt}")
                nc.vector.tensor_scalar_mul(w, masked, inv)
                wf.append(w)

            # ---- expert MLP ----
            def emit_stage_a(le):
                hT = hpool.tile([128, IT * T], mdt, tag="hT", name="hT")
                for (i0, ilen) in IGROUPS:
                    ps_a = [psa.tile([128, T], F32, tag="ps_a", name=f"ps_a{k}")
                            for k in range(ilen)]
                    for hc in range(2):
                        w1c = w1pool.tile([128, 8, 512], mdt, tag="w1c",
                                          name="w1c")
                        # expert 0 splits across SWDGE + sync: two parallel
                        # DMA paths during the cold-start head
                        eng = (nc.gpsimd if hc == 0 else nc.sync) \
                            if le == 0 else nc.sync
                        eng.dma_start(
                            out=w1c[:, :, :ilen * 128],
                            in_=w1_d[le, hc * 1024:(hc + 1) * 1024,
                                     i0 * 128:(i0 + ilen) * 128]
                            .rearrange("(j p) c -> p j c", p=128),
                        )
                        for j in range(8):
                            hk = hc * 8 + j
                            for itl in range(ilen):
                                nc.tensor.matmul(
                                    ps_a[itl],
                                    lhsT=w1c[:, j, itl * 128:(itl + 1) * 128],
                                    rhs=xt_sb[:, hk * T:(hk + 1) * T],
                                    start=hk == 0,
                                    stop=hk == HK - 1,
                                )
                    for itl in range(ilen):
                        it = i0 + itl
                        # silu(x) = x * sigmoid(x)  (CoreSim has no Silu table)
                        sg = rpool.tile([128, T], F32, tag="sg", name="sg")
                        nc.scalar.activation(sg, ps_a[itl],
                                             mybir.ActivationFunctionType.Sigmoid)
                        nc.vector.tensor_mul(hT[:, it * T:(it + 1) * T], sg,
                                             ps_a[itl])
                return hT

            def emit_stage_b(le, hT):
                for no in range(NO):
                    w2c = w2pool.tile([128, IT, 512], mdt, tag="w2c", name="w2c")
                    # second HWDGE ring (ACT queue) so w1/w2 streams parallelize
                    nc.scalar.dma_start(
                        out=w2c,
                        in_=w2_d[le, :, no * 512:(no + 1) * 512]
                        .rearrange("(j p) c -> p j c", p=128),
                    )
                    for tt in range(2):
                        ps_b = psb.tile([128, 512], F32, tag="ps_b", name="ps_b")
                        for ik in range(IT):
                            c0 = ik * T + tt * 128
                            nc.tensor.matmul(
                                ps_b,
                                lhsT=hT[:, c0:c0 + 128],
                                rhs=w2c[:, ik, :],
                                start=ik == 0,
                                stop=ik == IT - 1,
                            )
                        seg = acc[tt][:, no * 512:(no + 1) * 512]
                        nc.vector.scalar_tensor_tensor(
                            out=seg, in0=ps_b, scalar=wf[tt][:, le:le + 1],
                            in1=seg, op0=mybir.AluOpType.mult,
                            op1=mybir.AluOpType.add)
                        if le == EL - 1:
                            # last expert: stream each finished segment out
                            nc.sync.dma_start(
                                out=out_d[tt * 128:(tt + 1) * 128,
                                          no * 512:(no + 1) * 512],
                                in_=seg)

            # Expert 0's first matmuls only need the first w1 chunk + xt tiles,
            # so emit them before the router (which waits on the full xt32).
            hT0 = emit_stage_a(0)
            emit_router_inputs()
            emit_router(0)
            emit_router(1)
            emit_stage_b(0, hT0)
            for le in range(1, EL):
                hT = emit_stage_a(le)
                emit_stage_b(le, hT)


    nc.compile()
    return nc


def make_in_maps(x, gate_w, w1, w2, mdt=MDT):
    """Host-side sharding/layout prep. Returns one input dict per core."""
    npdt = _np_of(mdt)
    x = np.ascontiguousarray(np.asarray(x, np.float32))
    gate_w = np.ascontiguousarray(np.asarray(gate_w, np.float32))
    w1 = np.asarray(w1, np.float32)
    w2 = np.asarray(w2, np.float32)

    # [128, hk*T + t] = x[t, hk*128 + p]
    xt32 = np.ascontiguousarray(
        x.T.reshape(HK, 128, T).transpose(1, 0, 2).reshape(128, HK * T))
    xt = np.ascontiguousarray(xt32.astype(npdt))

    in_maps = []
    for c in range(NCORES):
        cols = list(range(c * EL, (c + 1) * EL)) + \
            [e for e in range(E) if not (c * EL <= e < (c + 1) * EL)]
        gperm = gate_w[:, cols]
        gate_t = np.ascontiguousarray(
            gperm.reshape(HK, 128, E).transpose(1, 0, 2).reshape(128, HK * E))
        in_maps.append({
            "xt": xt,
            "xt32": xt32,
            "gate": gate_t,
            "w1": np.ascontiguousarray(w1[c * EL:(c + 1) * EL].astype(npdt)),
            "w2": np.ascontiguousarray(w2[c * EL:(c + 1) * EL].astype(npdt)),
        })
    return in_maps


_NC_CACHE = {}


def _get_nc(mdt=MDT):
    if mdt not in _NC_CACHE:
        _NC_CACHE[mdt] = build(mdt)
    return _NC_CACHE[mdt]


def kernel(x, gate_w, w1, w2, topk=TOPK, **_):
    assert int(topk) == TOPK
    nc = _get_nc()
    in_maps = make_in_maps(x, gate_w, w1, w2)
    res = run_bass_kernel_spmd(nc, in_maps, core_ids=list(range(NCORES)))
    out = np.zeros((T, H), np.float32)
    for r in res.results:
        out += r["out"]
    return out

